# revision 31
# baseline (speedup 1.0000x reference)
"""Extended Kalman Filter kernel for 8 Trainium2 NeuronCores.

Math: the EKF covariance recursion (P -> A P A^T + Q; S = C P C^T + R;
K = P C^T S^-1; P -> (I-KC)P) does not depend on the data, only on cov0.
When cov0 is identical across the batch (it is: broadcast 0.1*I), the
per-timestep Kalman gains K_t are batch-independent and can be
precomputed on the host. The device-side work collapses to a linear
time-varying recursion on the mean only:

    mean_{t+1} = M_t @ mean_t + N_t @ u_t + K_t @ z_t
    M_t = (I - K_t C) A,  N_t = (I - K_t C) Bm,  dims D=6, U=6, O=3.

Device mapping (pure data-parallel over batch, 4096 batch/core):
  * The host pre-stages all tensor layouts (pure permutation + fp16
    cast, no math): batch on the matmul free dim, features on
    partitions, so the device does zero transposes. KG=14 batch groups
    per column -> state tiles [84, 294], input tiles [126, 294].
  * Everything runs in float16 (10-bit mantissa ~ tf32 for these O(1)
    values; measured rel err ~1.4e-3 vs the 2e-2 gate). All matmuls are
    single-pass 16-bit; PSUM accumulates fp32.
  * The serial bottleneck of a scan is the 64x (PE -> PSUM->SBUF copy
    -> PE) round trip. Two measures attack it:
      1. TIME-BLOCK SPLIT: steps 0..31 (chain A, seeded with mean0) and
         32..63 (chain B) run as two concurrent interleaved chains, so
         only 32 serial rounds remain on the wall clock.
      2. The seed state m_32 for chain B is computed ON DEVICE by 33
         independent matmuls accumulating into one PSUM bank:
         m_32 = kron(Phi_{32,0})^T' m0 + sum_s kron([Phi_{32,s+1}N_s;
         Phi K_s])^T' [u_s;z_s] -- the Phi are host-precomputed forward
         M-products (well-conditioned), and the rhs are the very same
         staged input tiles chain A consumes. These matmuls overlap
         chain A's early, latency-bound rounds and keep the PE
         continuously busy (high DVFS p-state).
  * Per chain round: input matmul (issued LOOKAHEAD ahead, independent)
    + state matmul accumulate in PSUM; one engine per chain (scalar=A,
    vector=B) copies PSUM -> fp16 collector tile (which IS the output:
    out dtype == state dtype). Chains never share SBUF tiles, PSUM
    banks, or copy engines, so no cross-engine serialization. Collector
    tiles hold 4 steps and are DMA'd in one descriptor-efficient burst.
"""

import numpy as np

T, BFULL, D, O, U = 64, 32768, 6, 3, 6
NCORES = 8
BS = BFULL // NCORES      # 4096 batch per core
KG = 14                   # batch groups per column (KG*(U+O)=126 <= 128)
COLS = 294                # ceil(4096/14) -> padded batch 4116
HC = COLS // 2            # column half for the split PSUM->SBUF copies
BP = KG * COLS            # 4116 padded batch per core
SR = KG * D               # 84 state rows
IR = KG * (U + O)         # 126 input rows
KT = 8                    # timesteps per input DMA staging group
NG = T // KT
OT = 4                    # timesteps per output collector tile
NO = T // OT
TBA = 36                  # chain A: steps 0..TBA-1
TBB = T - TBA             # chain B: steps TBA..63 (28)
LA = 2                    # input-matmul lookahead per chain
SEED_DELAY = TBA - TBB    # chain-A solo rounds; A and B then finish together

_CACHE = {}
LAST_RESULTS = None       # BassKernelResults of the most recent device run

F16 = np.float16


def _host_coeffs(cov0_row, A, Bm, Q_tril, C, R_tril):
    """Run the (batch-independent) covariance recursion on the host in
    float64; return per-step float32 coefficient matrices M_t, N_t, K_t."""
    A = np.asarray(A, np.float64)
    Bm = np.asarray(Bm, np.float64)
    Qt = np.asarray(Q_tril, np.float64)
    C = np.asarray(C, np.float64)
    Rt = np.asarray(R_tril, np.float64)
    Qc = Qt @ Qt.T
    Rc = Rt @ Rt.T
    P = np.asarray(cov0_row, np.float64)
    I = np.eye(D)
    Ms = np.empty((T, D, D), np.float32)
    Ns = np.empty((T, D, U), np.float32)
    Ks = np.empty((T, D, O), np.float32)
    for t in range(T):
        Pp = A @ P @ A.T + Qc
        S = C @ Pp @ C.T + Rc
        K = Pp @ C.T @ np.linalg.inv(S)
        IKC = I - K @ C
        Ms[t] = IKC @ A
        Ns[t] = IKC @ Bm
        Ks[t] = K
        P = IKC @ Pp
    return Ms, Ns, Ks


def _kron_lhsT(blocks_ji):
    """blocks_ji: [rows_per_group, T', cols_per_group(=D)] -> expanded
    block-diagonal lhsT [KG*rows, T', KG*D]."""
    r = blocks_ji.shape[0]
    tt = blocks_ji.shape[1]
    out = np.zeros((KG * r, tt, SR), F16)
    for g in range(KG):
        out[g * r:(g + 1) * r, :, g * D:(g + 1) * D] = blocks_ji
    return out


def _stat_arrays(Ms, Ns, Ks):
    """Chain stationaries (block-diagonal lhsT, time-grouped for DMA) and
    the seed-accumulation stationaries for m_32."""
    mt = np.transpose(Ms, (2, 0, 1)).astype(F16)          # [j,t,i] = M[t,i,j]
    nk = np.concatenate([np.transpose(Ns, (2, 0, 1)),
                         np.transpose(Ks, (2, 0, 1))], axis=0).astype(F16)
    ss = _kron_lhsT(mt)                                    # [SR, T, SR]
    isd = _kron_lhsT(nk)                                   # [IR, T, SR]
    ss = np.ascontiguousarray(ss.reshape(SR, NG, KT, SR).transpose(1, 0, 2, 3))
    isd = np.ascontiguousarray(isd.reshape(IR, NG, KT, SR).transpose(1, 0, 2, 3))

    # seed: m_TBA = Phi_{TBA,0} m0 + sum_{s<TBA} Phi_{TBA,s+1}(N_s u_s + K_s z_s)
    # with Phi_{TBA,s} = M_{TBA-1} @ ... @ M_s (forward products only)
    Ms64 = Ms.astype(np.float64)
    phis = [None] * (TBA + 1)
    acc = np.eye(D)
    phis[TBA] = acc
    for s in range(TBA - 1, -1, -1):
        acc = acc @ Ms64[s]
        phis[s] = acc
    seed_in = np.empty((9, TBA, D), np.float64)
    for s in range(TBA):
        fN = phis[s + 1] @ Ns.astype(np.float64)[s]        # [D, U]
        fK = phis[s + 1] @ Ks.astype(np.float64)[s]        # [D, O]
        seed_in[:U, s, :] = fN.T
        seed_in[U:, s, :] = fK.T
    seed_is = _kron_lhsT(seed_in.astype(F16))              # [IR, TBA, SR]
    seed_ss = _kron_lhsT(np.ascontiguousarray(
        phis[0].T.astype(F16))[:, None, :])                # [SR, 1, SR]
    return ss, isd, np.ascontiguousarray(seed_is), np.ascontiguousarray(seed_ss)


def _stage_inputs(meas_np, useq_np, mean0_np):
    """Feature-major fp16 staging of the per-core inputs (pure layout +
    dtype transform). Returns per-core lists (stage, m0s)."""
    stages, m0s = [], []
    for m in range(NCORES):
        sl = slice(m * BS, (m + 1) * BS)
        v = np.zeros((T, BP, U + O), np.float32)
        v[:, :BS, :U] = useq_np[:, sl]
        v[:, :BS, U:] = meas_np[:, sl]
        st = v.reshape(T, KG, COLS, U + O).transpose(0, 1, 3, 2)
        st = st.reshape(NG, KT, IR, COLS).transpose(0, 2, 1, 3)
        stages.append(np.ascontiguousarray(st).astype(F16))
        m0 = np.zeros((BP, D), np.float32)
        m0[:BS] = mean0_np[sl]
        m0s.append(np.ascontiguousarray(
            m0.reshape(KG, COLS, D).transpose(0, 2, 1).reshape(SR, COLS))
            .astype(F16))
    return stages, m0s


def _build_program():
    """Build (once) the Bass/Tile program shared by all 8 cores."""
    if "nc" in _CACHE:
        return _CACHE["nc"]

    import concourse.bacc as bacc
    import concourse.tile as tile
    from concourse import mybir

    f16 = mybir.dt.float16
    f32 = mybir.dt.float32
    nc = bacc.Bacc("TRN2", target_bir_lowering=False, debug=False,
                   num_devices=NCORES)

    stage = nc.dram_tensor("stage", [NG, IR, KT, COLS], f16,
                           kind="ExternalInput").ap()
    m0d = nc.dram_tensor("m0s", [SR, COLS], f16, kind="ExternalInput").ap()
    ssd = nc.dram_tensor("ss", [NG, SR, KT, SR], f16,
                         kind="ExternalInput").ap()
    isd = nc.dram_tensor("is", [NG, IR, KT, SR], f16,
                         kind="ExternalInput").ap()
    sisd = nc.dram_tensor("sis", [IR, TBA, SR], f16, kind="ExternalInput").ap()
    sssd = nc.dram_tensor("sss", [SR, 1, SR], f16, kind="ExternalInput").ap()
    out = nc.dram_tensor("out", [NO, SR, OT, COLS], f16,
                         kind="ExternalOutput").ap()

    with tile.TileContext(nc) as tc:
        with (
            tc.tile_pool(name="const", bufs=1) as const,
            tc.tile_pool(name="stage", bufs=8) as stg,
            tc.tile_pool(name="collA", bufs=4) as collpA,
            tc.tile_pool(name="collB", bufs=4) as collpB,
            tc.tile_pool(name="psA", bufs=LA + 1, space="PSUM") as pspA,
            tc.tile_pool(name="psB", bufs=LA + 1, space="PSUM") as pspB,
            tc.tile_pool(name="psS", bufs=1, space="PSUM") as pspS,
            tc.tile_pool(name="psD", bufs=1, space="PSUM") as pspD,
        ):
            ss_t = const.tile([SR, T, SR], f16)
            is_t = const.tile([IR, T, SR], f16)
            sis_t = const.tile([IR, TBA, SR], f16)
            sss_t = const.tile([SR, 1, SR], f16)
            st_init = const.tile([SR, COLS], f16)
            seed_t = const.tile([SR, COLS], f16)
            nc.scalar.dma_start(sss_t[:], sssd[:])
            nc.scalar.dma_start(st_init[:], m0d[:])
            nc.scalar.dma_start(sis_t[:, 0:TBA // 2, :],
                                sisd[:, 0:TBA // 2, :])
            nc.scalar.dma_start(sis_t[:, TBA // 2:TBA, :],
                                sisd[:, TBA // 2:TBA, :])

            ins = {}

            def load_group(g):
                it = stg.tile([IR, KT, COLS], f16, tag="in",
                              name=f"in_{g}", bufs=8)
                nc.sync.dma_start(it[:], stage[g])
                ins[g] = it

            def load_stats(g):
                ts = slice(g * KT, (g + 1) * KT)
                nc.sync.dma_start(ss_t[:, ts, :], ssd[g])
                nc.sync.dma_start(is_t[:, ts, :], isd[g])

            # sequential: chain A + the seed consume groups 0..4 in order,
            # chain B picks up from group 4 once the seed is done
            for g in range(NG):
                load_group(g)
                load_stats(g)

            psbA, psbB = {}, {}
            collsA, collsB = {}, {}

            def in_mm(chain, r):
                """Input-injection matmul for chain round r (independent)."""
                t = r if chain == 0 else TBA + r
                g, tl = divmod(t, KT)
                psp = pspA if chain == 0 else pspB
                ps = psp.tile([SR, COLS], f32, tag=f"ps{chain}",
                              name=f"ps{chain}_{r}", bufs=LA + 1)
                nc.tensor.matmul(ps[:], is_t[:, t, :], ins[g][:, tl, :],
                                 start=True, stop=False)
                (psbA if chain == 0 else psbB)[r] = ps

            # seed accumulation state
            seed_ps = pspS.tile([SR, COLS], f32, tag="psS", bufs=1)
            # dummy-matmul target: keeps the PE continuously busy through
            # the latency-bound pair phase so it holds its high DVFS
            # p-state (the PE down-clocks whenever it idles)
            dummy_ps = pspD.tile([SR, COLS], f32, tag="psD", bufs=1)
            dummy_on = [False]
            seed_emitted = [0]
            seed_m0_done = [False]

            def emit_seed(n):
                if not seed_m0_done[0]:
                    nc.tensor.matmul(seed_ps[:], sss_t[:, 0, :], st_init[:],
                                     start=True, stop=False)
                    seed_m0_done[0] = True
                for _ in range(n):
                    s = seed_emitted[0]
                    if s >= TBA:
                        return
                    g, tl = divmod(s, KT)
                    nc.tensor.matmul(seed_ps[:], sis_t[:, s, :],
                                     ins[g][:, tl, :],
                                     start=False, stop=(s == TBA - 1))
                    seed_emitted[0] += 1
                    if s == TBA - 1:
                        # seed copy: vector (chain B's engine, idle until
                        # B's first round, which depends on this anyway)
                        nc.vector.tensor_copy(seed_t[:], seed_ps[:])

            def round_(chain, r):
                """One serial round of a chain: state matmul + copy."""
                t = r if chain == 0 else TBA + r
                nsteps = TBA if chain == 0 else TBB
                ot, tl = divmod(t, OT)
                colls = collsA if chain == 0 else collsB
                collp = collpA if chain == 0 else collpB
                if tl == 0:
                    colls[ot] = collp.tile([SR, OT, COLS], f16,
                                           tag=f"coll{chain}",
                                           name=f"coll{chain}_{ot}", bufs=4)
                if r == 0:
                    prev = st_init[:] if chain == 0 else seed_t[:]
                else:
                    prev = colls[(t - 1) // OT][:, (t - 1) % OT, :]
                ps = (psbA if chain == 0 else psbB).pop(r)
                nc.tensor.matmul(ps[:], ss_t[:, t, :], prev,
                                 start=False, stop=True)
                if r + LA < nsteps:
                    in_mm(chain, r + LA)
                cur = colls[ot][:, tl, :]
                # split each copy across both PSUM-capable engines so the
                # serial round only pays ~half a copy of latency
                if chain == 0:
                    nc.scalar.copy(cur[:, 0:HC], ps[:, 0:HC])
                    nc.vector.tensor_copy(cur[:, HC:COLS], ps[:, HC:COLS])
                else:
                    nc.vector.tensor_copy(cur[:, 0:HC], ps[:, 0:HC])
                    nc.scalar.copy(cur[:, HC:COLS], ps[:, HC:COLS])
                if dummy_on[0]:
                    nc.tensor.matmul(dummy_ps[:], ss_t[:, 0, :], st_init[:],
                                     start=True, stop=True,
                                     skip_group_check=True)
                if tl == OT - 1:
                    dma_eng = nc.scalar if chain == 0 else nc.sync
                    dma_eng.dma_start(out[ot], colls[ot][:])

            for r in range(LA):
                in_mm(0, r)

            # chain A runs SEED_DELAY rounds solo (seed matmuls fill the
            # PE between its latency-bound rounds); B's lookahead input
            # matmuls are only emitted at pair-up so they never block the
            # in-order PE queue on not-yet-loaded input groups. A and B
            # then run paired and finish together (TBA - TBB = delay).
            for r in range(SEED_DELAY):
                round_(0, r)
                emit_seed(5)
            emit_seed(TBA)   # any remainder
            for r in range(LA):
                in_mm(1, r)
            dummy_on[0] = True
            for r in range(SEED_DELAY, TBA):
                round_(0, r)
                round_(1, r - SEED_DELAY)

    nc.compile()
    _CACHE["nc"] = nc
    return nc


def _run_device(meas_np, useq_np, mean0_np, Ms, Ns, Ks, trace=False):
    global LAST_RESULTS
    from concourse import bass_utils

    nc = _build_program()
    ss, isd, seed_is, seed_ss = _stat_arrays(Ms, Ns, Ks)
    stages, m0s = _stage_inputs(meas_np, useq_np, mean0_np)
    in_maps = []
    for m in range(NCORES):
        in_maps.append({
            "stage": stages[m], "m0s": m0s[m], "ss": ss, "is": isd,
            "sis": seed_is, "sss": seed_ss,
        })
    res = bass_utils.run_bass_kernel_spmd(
        nc, in_maps, core_ids=list(range(NCORES)), trace=trace)
    LAST_RESULTS = res
    outs = []
    for m in range(NCORES):
        o = np.asarray(res.results[m]["out"]).astype(np.float32)
        o = o.reshape(NO, KG, D, OT, COLS).transpose(0, 3, 1, 4, 2)
        outs.append(o.reshape(T, BP, D)[:, :BS])
    return np.concatenate(outs, axis=1)


def _numpy_fallback(measurements, inputs_seq, mean0, cov0, A, Bm, Q_tril, C, R_tril):
    """General (per-batch covariance) EKF in vectorized numpy. Correctness
    fallback only; used when cov0 is not batch-uniform."""
    f = np.float32
    A = np.asarray(A, f); Bm = np.asarray(Bm, f); C = np.asarray(C, f)
    Qc = (np.asarray(Q_tril, f) @ np.asarray(Q_tril, f).T).astype(f)
    Rc = (np.asarray(R_tril, f) @ np.asarray(R_tril, f).T).astype(f)
    mean = np.asarray(mean0, f).copy()
    cov = np.asarray(cov0, f).copy()
    I = np.eye(D, dtype=f)
    outs = np.empty((T, mean.shape[0], D), f)
    for t in range(T):
        z = np.asarray(measurements[t], f)
        u = np.asarray(inputs_seq[t], f)
        pm = mean @ A.T + u @ Bm.T
        pc = np.einsum('ij,bjk,lk->bil', A, cov, A) + Qc
        innov = z - pm @ C.T
        S = np.einsum('ij,bjk,lk->bil', C, pc, C) + Rc
        PCt = np.einsum('bij,kj->bik', pc, C)
        K = PCt @ np.linalg.inv(S)
        mean = pm + np.einsum('bij,bj->bi', K, innov)
        cov = (I - np.einsum('bij,jk->bik', K, C)) @ pc
        outs[t] = mean
    return outs


def kernel(measurements, inputs_seq, mean0, cov0, A, Bm, Q_tril, C, R_tril):
    measurements = np.asarray(measurements)
    inputs_seq = np.asarray(inputs_seq)
    mean0 = np.asarray(mean0)
    cov0 = np.asarray(cov0)

    if np.ptp(cov0, axis=0).max() != 0.0:
        return _numpy_fallback(measurements, inputs_seq, mean0, cov0,
                               A, Bm, Q_tril, C, R_tril)

    Ms, Ns, Ks = _host_coeffs(cov0[0], A, Bm, Q_tril, C, R_tril)
    return _run_device(measurements.astype(np.float32),
                       inputs_seq.astype(np.float32),
                       mean0.astype(np.float32), Ms, Ns, Ks,
                       trace=False)


# revision 36
# speedup vs baseline: 1.2162x; 1.2162x over previous
"""Extended Kalman Filter kernel for 8 Trainium2 NeuronCores.

Math: the EKF covariance recursion (P -> A P A^T + Q; S = C P C^T + R;
K = P C^T S^-1; P -> (I-KC)P) does not depend on the data, only on cov0.
When cov0 is identical across the batch (it is: broadcast 0.1*I), the
per-timestep Kalman gains K_t are batch-independent and can be
precomputed on the host. The device-side work collapses to a linear
time-varying recursion on the mean only:

    mean_{t+1} = M_t @ mean_t + N_t @ u_t + K_t @ z_t
    M_t = (I - K_t C) A,  N_t = (I - K_t C) Bm,  dims D=6, U=6, O=3.

Device mapping (pure data-parallel over batch, 4096 batch/core):
  * The host pre-stages all tensor layouts (pure permutation + fp16
    cast, no math): batch on the matmul free dim, features on
    partitions, so the device does zero transposes. KG=14 batch groups
    per column -> state tiles [84, 294], input tiles [126, 294].
  * Everything runs in float16 (10-bit mantissa ~ tf32 for these O(1)
    values; measured rel err ~1.4e-3 vs the 2e-2 gate). All matmuls are
    single-pass 16-bit; PSUM accumulates fp32.
  * The serial bottleneck of a scan is the 64x (PE -> PSUM->SBUF copy
    -> PE) round trip. Two measures attack it:
      1. TIME-BLOCK SPLIT: steps 0..31 (chain A, seeded with mean0) and
         32..63 (chain B) run as two concurrent interleaved chains, so
         only 32 serial rounds remain on the wall clock.
      2. The seed state m_32 for chain B is computed ON DEVICE by 33
         independent matmuls accumulating into one PSUM bank:
         m_32 = kron(Phi_{32,0})^T' m0 + sum_s kron([Phi_{32,s+1}N_s;
         Phi K_s])^T' [u_s;z_s] -- the Phi are host-precomputed forward
         M-products (well-conditioned), and the rhs are the very same
         staged input tiles chain A consumes. These matmuls overlap
         chain A's early, latency-bound rounds and keep the PE
         continuously busy (high DVFS p-state).
  * Per chain round: input matmul (issued LOOKAHEAD ahead, independent)
    + state matmul accumulate in PSUM; one engine per chain (scalar=A,
    vector=B) copies PSUM -> fp16 collector tile (which IS the output:
    out dtype == state dtype). Chains never share SBUF tiles, PSUM
    banks, or copy engines, so no cross-engine serialization. Collector
    tiles hold 4 steps and are DMA'd in one descriptor-efficient burst.
"""

import numpy as np

T, BFULL, D, O, U = 64, 32768, 6, 3, 6
NCORES = 8
BS = BFULL // NCORES      # 4096 batch per core
KG = 14                   # batch groups per column (KG*(U+O)=126 <= 128)
COLS = 294                # ceil(4096/14) -> padded batch 4116
HC = COLS // 2            # column half for the split PSUM->SBUF copies
BP = KG * COLS            # 4116 padded batch per core
SR = KG * D               # 84 state rows
IR = KG * (U + O)         # 126 input rows
KT = 8                    # timesteps per input DMA staging group
NG = T // KT
OT = 4                    # timesteps per output collector tile
NO = T // OT
TBA = 36                  # chain A: steps 0..TBA-1
TBB = T - TBA             # chain B: steps TBA..63 (28)
LA = 2                    # input-matmul lookahead per chain
SEED_DELAY = TBA - TBB    # chain-A solo rounds; A and B then finish together

_CACHE = {}
LAST_RESULTS = None       # BassKernelResults of the most recent device run

F16 = np.float16


def _host_coeffs(cov0_row, A, Bm, Q_tril, C, R_tril):
    """Run the (batch-independent) covariance recursion on the host in
    float64; return per-step float32 coefficient matrices M_t, N_t, K_t."""
    A = np.asarray(A, np.float64)
    Bm = np.asarray(Bm, np.float64)
    Qt = np.asarray(Q_tril, np.float64)
    C = np.asarray(C, np.float64)
    Rt = np.asarray(R_tril, np.float64)
    Qc = Qt @ Qt.T
    Rc = Rt @ Rt.T
    P = np.asarray(cov0_row, np.float64)
    I = np.eye(D)
    Ms = np.empty((T, D, D), np.float32)
    Ns = np.empty((T, D, U), np.float32)
    Ks = np.empty((T, D, O), np.float32)
    for t in range(T):
        Pp = A @ P @ A.T + Qc
        S = C @ Pp @ C.T + Rc
        K = Pp @ C.T @ np.linalg.inv(S)
        IKC = I - K @ C
        Ms[t] = IKC @ A
        Ns[t] = IKC @ Bm
        Ks[t] = K
        P = IKC @ Pp
    return Ms, Ns, Ks


def _kron_lhsT(blocks_ji):
    """blocks_ji: [rows_per_group, T', cols_per_group(=D)] -> expanded
    block-diagonal lhsT [KG*rows, T', KG*D]."""
    r = blocks_ji.shape[0]
    tt = blocks_ji.shape[1]
    out = np.zeros((KG * r, tt, SR), F16)
    for g in range(KG):
        out[g * r:(g + 1) * r, :, g * D:(g + 1) * D] = blocks_ji
    return out


def _stat_arrays(Ms, Ns, Ks):
    """Chain stationaries (block-diagonal lhsT, time-grouped for DMA) and
    the seed-accumulation stationaries for m_32."""
    mt = np.transpose(Ms, (2, 0, 1)).astype(F16)          # [j,t,i] = M[t,i,j]
    nk = np.concatenate([np.transpose(Ns, (2, 0, 1)),
                         np.transpose(Ks, (2, 0, 1))], axis=0).astype(F16)
    ss = _kron_lhsT(mt)                                    # [SR, T, SR]
    isd = _kron_lhsT(nk)                                   # [IR, T, SR]
    ss = np.ascontiguousarray(ss.reshape(SR, NG, KT, SR).transpose(1, 0, 2, 3))
    isd = np.ascontiguousarray(isd.reshape(IR, NG, KT, SR).transpose(1, 0, 2, 3))

    # seed: m_TBA = Phi_{TBA,0} m0 + sum_{s<TBA} Phi_{TBA,s+1}(N_s u_s + K_s z_s)
    # with Phi_{TBA,s} = M_{TBA-1} @ ... @ M_s (forward products only)
    Ms64 = Ms.astype(np.float64)
    phis = [None] * (TBA + 1)
    acc = np.eye(D)
    phis[TBA] = acc
    for s in range(TBA - 1, -1, -1):
        acc = acc @ Ms64[s]
        phis[s] = acc
    seed_in = np.empty((9, TBA, D), np.float64)
    for s in range(TBA):
        fN = phis[s + 1] @ Ns.astype(np.float64)[s]        # [D, U]
        fK = phis[s + 1] @ Ks.astype(np.float64)[s]        # [D, O]
        seed_in[:U, s, :] = fN.T
        seed_in[U:, s, :] = fK.T
    seed_is = _kron_lhsT(seed_in.astype(F16))              # [IR, TBA, SR]
    seed_ss = _kron_lhsT(np.ascontiguousarray(
        phis[0].T.astype(F16))[:, None, :])                # [SR, 1, SR]
    return ss, isd, np.ascontiguousarray(seed_is), np.ascontiguousarray(seed_ss)


def _stage_inputs(meas_np, useq_np, mean0_np):
    """Feature-major fp16 staging of the per-core inputs (pure layout +
    dtype transform). Returns per-core lists (stage, m0s)."""
    stages, m0s = [], []
    for m in range(NCORES):
        sl = slice(m * BS, (m + 1) * BS)
        v = np.zeros((T, BP, U + O), np.float32)
        v[:, :BS, :U] = useq_np[:, sl]
        v[:, :BS, U:] = meas_np[:, sl]
        st = v.reshape(T, KG, COLS, U + O).transpose(0, 1, 3, 2)
        st = st.reshape(NG, KT, IR, COLS).transpose(0, 2, 1, 3)
        stages.append(np.ascontiguousarray(st).astype(F16))
        m0 = np.zeros((BP, D), np.float32)
        m0[:BS] = mean0_np[sl]
        m0s.append(np.ascontiguousarray(
            m0.reshape(KG, COLS, D).transpose(0, 2, 1).reshape(SR, COLS))
            .astype(F16))
    return stages, m0s


def _build_program():
    """Build (once) the Bass/Tile program shared by all 8 cores."""
    if "nc" in _CACHE:
        return _CACHE["nc"]

    import concourse.bacc as bacc
    import concourse.tile as tile
    from concourse import mybir

    f16 = mybir.dt.float16
    f32 = mybir.dt.float32
    nc = bacc.Bacc("TRN2", target_bir_lowering=False, debug=False,
                   num_devices=NCORES)

    stage = nc.dram_tensor("stage", [NG, IR, KT, COLS], f16,
                           kind="ExternalInput").ap()
    m0d = nc.dram_tensor("m0s", [SR, COLS], f16, kind="ExternalInput").ap()
    ssd = nc.dram_tensor("ss", [NG, SR, KT, SR], f16,
                         kind="ExternalInput").ap()
    isd = nc.dram_tensor("is", [NG, IR, KT, SR], f16,
                         kind="ExternalInput").ap()
    sisd = nc.dram_tensor("sis", [IR, TBA, SR], f16, kind="ExternalInput").ap()
    sssd = nc.dram_tensor("sss", [SR, 1, SR], f16, kind="ExternalInput").ap()
    out = nc.dram_tensor("out", [NO, SR, OT, COLS], f16,
                         kind="ExternalOutput").ap()

    with tile.TileContext(nc) as tc:
        with (
            tc.tile_pool(name="const", bufs=1) as const,
            tc.tile_pool(name="stage", bufs=8) as stg,
            tc.tile_pool(name="collA", bufs=4) as collpA,
            tc.tile_pool(name="collB", bufs=4) as collpB,
            tc.tile_pool(name="psA", bufs=LA + 1, space="PSUM") as pspA,
            tc.tile_pool(name="psB", bufs=LA + 1, space="PSUM") as pspB,
            tc.tile_pool(name="psS", bufs=1, space="PSUM") as pspS,
        ):
            ss_t = const.tile([SR, T, SR], f16)
            is_t = const.tile([IR, T, SR], f16)
            sis_t = const.tile([IR, TBA, SR], f16)
            sss_t = const.tile([SR, 1, SR], f16)
            st_init = const.tile([SR, COLS], f16)
            seed_t = const.tile([SR, COLS], f16)
            nc.scalar.dma_start(sss_t[:], sssd[:])
            nc.scalar.dma_start(st_init[:], m0d[:])
            nc.scalar.dma_start(sis_t[:, 0:TBA // 2, :],
                                sisd[:, 0:TBA // 2, :])
            nc.scalar.dma_start(sis_t[:, TBA // 2:TBA, :],
                                sisd[:, TBA // 2:TBA, :])

            ins = {}

            def load_group(g, split=False):
                it = stg.tile([IR, KT, COLS], f16, tag="in",
                              name=f"in_{g}", bufs=8)
                if split:
                    h = KT // 2
                    nc.sync.dma_start(it[:, 0:h, :], stage[g, :, 0:h, :])
                    nc.sync.dma_start(it[:, h:KT, :], stage[g, :, h:KT, :])
                else:
                    nc.sync.dma_start(it[:], stage[g])
                ins[g] = it

            def load_stats(g):
                ts = slice(g * KT, (g + 1) * KT)
                nc.sync.dma_start(is_t[:, ts, :], isd[g])
                nc.sync.dma_start(ss_t[:, ts, :], ssd[g])

            # sequential: chain A + the seed consume groups 0..4 in order,
            # chain B picks up from group 4 once the seed is done. The
            # first group's staging DMA is split in two so the first input
            # matmuls start as soon as the first half lands.
            for g in range(NG):
                load_group(g, split=(g == 0))
                load_stats(g)

            psbA, psbB = {}, {}
            collsA, collsB = {}, {}

            def in_mm(chain, r):
                """Input-injection matmul for chain round r (independent)."""
                t = r if chain == 0 else TBA + r
                g, tl = divmod(t, KT)
                psp = pspA if chain == 0 else pspB
                ps = psp.tile([SR, COLS], f32, tag=f"ps{chain}",
                              name=f"ps{chain}_{r}", bufs=LA + 1)
                nc.tensor.matmul(ps[:], is_t[:, t, :], ins[g][:, tl, :],
                                 start=True, stop=False)
                (psbA if chain == 0 else psbB)[r] = ps

            # seed accumulation state
            seed_ps = pspS.tile([SR, COLS], f32, tag="psS", bufs=1)
            seed_emitted = [0]
            seed_m0_done = [False]

            def emit_seed(n):
                if not seed_m0_done[0]:
                    nc.tensor.matmul(seed_ps[:], sss_t[:, 0, :], st_init[:],
                                     start=True, stop=False)
                    seed_m0_done[0] = True
                for _ in range(n):
                    s = seed_emitted[0]
                    if s >= TBA:
                        return
                    g, tl = divmod(s, KT)
                    nc.tensor.matmul(seed_ps[:], sis_t[:, s, :],
                                     ins[g][:, tl, :],
                                     start=False, stop=(s == TBA - 1))
                    seed_emitted[0] += 1
                    if s == TBA - 1:
                        # seed copy: vector (chain B's engine, idle until
                        # B's first round, which depends on this anyway)
                        nc.vector.tensor_copy(seed_t[:], seed_ps[:])

            def round_(chain, r):
                """One serial round of a chain: state matmul + copy."""
                t = r if chain == 0 else TBA + r
                nsteps = TBA if chain == 0 else TBB
                ot, tl = divmod(t, OT)
                colls = collsA if chain == 0 else collsB
                collp = collpA if chain == 0 else collpB
                if tl == 0:
                    colls[ot] = collp.tile([SR, OT, COLS], f16,
                                           tag=f"coll{chain}",
                                           name=f"coll{chain}_{ot}", bufs=4)
                if r == 0:
                    prev = st_init[:] if chain == 0 else seed_t[:]
                else:
                    prev = colls[(t - 1) // OT][:, (t - 1) % OT, :]
                ps = (psbA if chain == 0 else psbB).pop(r)
                nc.tensor.matmul(ps[:], ss_t[:, t, :], prev,
                                 start=False, stop=True)
                if r + LA < nsteps:
                    in_mm(chain, r + LA)
                cur = colls[ot][:, tl, :]
                # one whole-tile copy per chain on its own engine: Tile
                # serializes different-engine writes to a shared tile, so
                # half-splits across engines do not overlap
                if chain == 0:
                    nc.scalar.copy(cur[:], ps[:])
                else:
                    nc.vector.tensor_copy(cur[:], ps[:])
                if tl == OT - 1:
                    dma_eng = nc.scalar if chain == 0 else nc.sync
                    dma_eng.dma_start(out[ot], colls[ot][:])

            for r in range(LA):
                in_mm(0, r)

            # chain A runs SEED_DELAY rounds solo (seed matmuls fill the
            # PE between its latency-bound rounds); B's lookahead input
            # matmuls are only emitted at pair-up so they never block the
            # in-order PE queue on not-yet-loaded input groups. A and B
            # then run paired and finish together (TBA - TBB = delay).
            for r in range(SEED_DELAY):
                round_(0, r)
                emit_seed(5)
            emit_seed(TBA)   # any remainder
            for r in range(LA):
                in_mm(1, r)
            for r in range(SEED_DELAY, TBA):
                round_(0, r)
                round_(1, r - SEED_DELAY)

    nc.compile()
    _CACHE["nc"] = nc
    return nc


def _run_device(meas_np, useq_np, mean0_np, Ms, Ns, Ks, trace=False):
    global LAST_RESULTS
    from concourse import bass_utils

    nc = _build_program()
    ss, isd, seed_is, seed_ss = _stat_arrays(Ms, Ns, Ks)
    stages, m0s = _stage_inputs(meas_np, useq_np, mean0_np)
    in_maps = []
    for m in range(NCORES):
        in_maps.append({
            "stage": stages[m], "m0s": m0s[m], "ss": ss, "is": isd,
            "sis": seed_is, "sss": seed_ss,
        })
    res = bass_utils.run_bass_kernel_spmd(
        nc, in_maps, core_ids=list(range(NCORES)), trace=trace)
    LAST_RESULTS = res
    outs = []
    for m in range(NCORES):
        o = np.asarray(res.results[m]["out"]).astype(np.float32)
        o = o.reshape(NO, KG, D, OT, COLS).transpose(0, 3, 1, 4, 2)
        outs.append(o.reshape(T, BP, D)[:, :BS])
    return np.concatenate(outs, axis=1)


def _numpy_fallback(measurements, inputs_seq, mean0, cov0, A, Bm, Q_tril, C, R_tril):
    """General (per-batch covariance) EKF in vectorized numpy. Correctness
    fallback only; used when cov0 is not batch-uniform."""
    f = np.float32
    A = np.asarray(A, f); Bm = np.asarray(Bm, f); C = np.asarray(C, f)
    Qc = (np.asarray(Q_tril, f) @ np.asarray(Q_tril, f).T).astype(f)
    Rc = (np.asarray(R_tril, f) @ np.asarray(R_tril, f).T).astype(f)
    mean = np.asarray(mean0, f).copy()
    cov = np.asarray(cov0, f).copy()
    I = np.eye(D, dtype=f)
    outs = np.empty((T, mean.shape[0], D), f)
    for t in range(T):
        z = np.asarray(measurements[t], f)
        u = np.asarray(inputs_seq[t], f)
        pm = mean @ A.T + u @ Bm.T
        pc = np.einsum('ij,bjk,lk->bil', A, cov, A) + Qc
        innov = z - pm @ C.T
        S = np.einsum('ij,bjk,lk->bil', C, pc, C) + Rc
        PCt = np.einsum('bij,kj->bik', pc, C)
        K = PCt @ np.linalg.inv(S)
        mean = pm + np.einsum('bij,bj->bi', K, innov)
        cov = (I - np.einsum('bij,jk->bik', K, C)) @ pc
        outs[t] = mean
    return outs


def kernel(measurements, inputs_seq, mean0, cov0, A, Bm, Q_tril, C, R_tril):
    measurements = np.asarray(measurements)
    inputs_seq = np.asarray(inputs_seq)
    mean0 = np.asarray(mean0)
    cov0 = np.asarray(cov0)

    if np.ptp(cov0, axis=0).max() != 0.0:
        return _numpy_fallback(measurements, inputs_seq, mean0, cov0,
                               A, Bm, Q_tril, C, R_tril)

    Ms, Ns, Ks = _host_coeffs(cov0[0], A, Bm, Q_tril, C, R_tril)
    return _run_device(measurements.astype(np.float32),
                       inputs_seq.astype(np.float32),
                       mean0.astype(np.float32), Ms, Ns, Ks,
                       trace=False)
